# revision 5
# baseline (speedup 1.0000x reference)
"""Multi-head attention on 8 TRN2 NeuronCores (Bass/Tile, SPMD, no collectives).

Problem: B=4, Sf=St=2048, DIM=768, H=12, Dh=64, f32 reference.

Sharding: (batch, Sf/2) -> 8 shards. Core c handles batch b=c//2, query rows
[512*(c%2)*2 : +1024). K/V projections for a batch are recomputed by both cores
of the pair (cheaper than any collective).

Device dataflow is fully transposed so no on-chip transposes are needed:
  QT[h]  [64,1024]  = Wq_h^T @ xf^T        (lhsT=Wq cols, rhs=xf^T)
  KT[h]  [64,2048]  = Wk_h^T @ xt^T
  V      [2048,780] = xt_aug^T^T @ Wv_aug  (per-head ones column folded in)
  S^T    [St,Sq]    = KT_chunk^T^T ... per (head, st-tile): lhsT=KT[64,128], rhs=QT[64,1024]
  P^T    = exp(S^T) * mask^T               (no max-subtract; masked lanes exp->*0)
  ctx^T/Z[65,1024]  = [V_h|1]^T @ P^T      (accumulated over 16 st-tiles in PSUM)
  out^T  [768,1024] = Wo^T @ (ctx^T * 1/Z) + bo
Host transposes out^T back and stitches the 8 shards.
"""

import os
import numpy as np
import ml_dtypes

BF16 = ml_dtypes.bfloat16

B, SF, ST, DIM = 4, 2048, 2048, 768
NH, HD = 12, 64
SCALE = HD ** -0.5
NCORES = 8
ROWS = B * SF // NCORES      # 1024 query rows per core
HP = NH // 2                 # 6 head-pairs == 6 x 128-partition chunks of DIM
VW = NH * (HD + 1)           # 780: V width with per-head ones columns
NST = ST // 128              # 16 st tiles

_CACHED_NC = None


def _build_nc():
    from concourse import bacc, tile, mybir
    import concourse.bass as bass

    dt = mybir.dt
    nc = bacc.Bacc("TRN2", target_bir_lowering=False, debug=False,
                   num_devices=NCORES)

    xfT = nc.dram_tensor("xfT", [DIM, ROWS], dt.bfloat16, kind="ExternalInput").ap()
    xtT = nc.dram_tensor("xtT", [DIM, ST], dt.bfloat16, kind="ExternalInput").ap()
    maskT = nc.dram_tensor("maskT", [ST, ROWS], dt.bfloat16, kind="ExternalInput").ap()
    wq = nc.dram_tensor("wq", [DIM, DIM], dt.bfloat16, kind="ExternalInput").ap()
    wk = nc.dram_tensor("wk", [DIM, DIM], dt.bfloat16, kind="ExternalInput").ap()
    wv = nc.dram_tensor("wv", [DIM + 1, VW], dt.bfloat16, kind="ExternalInput").ap()
    wo = nc.dram_tensor("wo", [DIM, DIM], dt.bfloat16, kind="ExternalInput").ap()
    biases = nc.dram_tensor("biases", [128, 3 * HP], dt.float32, kind="ExternalInput").ap()
    out = nc.dram_tensor("out", [DIM, ROWS], dt.float32, kind="ExternalOutput").ap()
    rz_dram = nc.dram_tensor("rz_scratch", [NH, ROWS], dt.float32).ap()

    EXP = mybir.ActivationFunctionType.Exp

    with tile.TileContext(nc) as tc:
        persist_cm = tc.tile_pool(name="persist", bufs=1)
        persist = persist_cm.__enter__()

        wo_sb = []
        for k in range(HP):
            t = persist.tile([128, DIM], dt.bfloat16, tag=f"wo{k}", name=f"wo{k}")
            nc.sync.dma_start(out=t, in_=wo[k * 128:(k + 1) * 128, :])
            wo_sb.append(t)
        bias_sb = persist.tile([128, 3 * HP], dt.float32, tag="biases", name="biases")
        nc.sync.dma_start(out=bias_sb, in_=biases)

        qt_sb = [persist.tile([128, ROWS], dt.bfloat16, tag=f"qt{i}", name=f"qt{i}") for i in range(HP)]
        kt_sb = [persist.tile([128, ST], dt.bfloat16, tag=f"kt{i}", name=f"kt{i}") for i in range(HP)]
        v_sb = [persist.tile([128, VW], dt.bfloat16, tag=f"v{i}", name=f"v{i}") for i in range(NST)]
        ctx_sb = [persist.tile([128, ROWS], dt.bfloat16, tag=f"ctx{i}", name=f"ctx{i}") for i in range(HP)]

        # ---------------- phase A: projections ----------------
        with tc.tile_pool(name="projIn", bufs=1) as projin, \
             tc.tile_pool(name="psA", bufs=4, space="PSUM") as psA:
            xf_sb, xt_sb, wq_sb, wk_sb, wv_sb = [], [], [], [], []
            for k in range(HP):
                t = projin.tile([128, ROWS], dt.bfloat16, tag=f"xf{k}", name=f"xf{k}")
                nc.sync.dma_start(out=t, in_=xfT[k * 128:(k + 1) * 128, :])
                xf_sb.append(t)
                t = projin.tile([128, DIM], dt.bfloat16, tag=f"wq{k}", name=f"wq{k}")
                nc.sync.dma_start(out=t, in_=wq[k * 128:(k + 1) * 128, :])
                wq_sb.append(t)
            for k in range(HP):
                t = projin.tile([128, ST], dt.bfloat16, tag=f"xt{k}", name=f"xt{k}")
                nc.sync.dma_start(out=t, in_=xtT[k * 128:(k + 1) * 128, :])
                xt_sb.append(t)
                t = projin.tile([128, DIM], dt.bfloat16, tag=f"wk{k}", name=f"wk{k}")
                nc.sync.dma_start(out=t, in_=wk[k * 128:(k + 1) * 128, :])
                wk_sb.append(t)
                t = projin.tile([128, VW], dt.bfloat16, tag=f"wv{k}", name=f"wv{k}")
                nc.sync.dma_start(out=t, in_=wv[k * 128:(k + 1) * 128, :])
                wv_sb.append(t)
            wv_bias = projin.tile([1, VW], dt.bfloat16, tag="wvb", name="wvb")
            nc.sync.dma_start(out=wv_bias, in_=wv[DIM:DIM + 1, :])
            ones_sb = projin.tile([1, ST], dt.bfloat16, tag="ones", name="ones")
            nc.vector.memset(ones_sb, 1.0)

            # QT: per head-pair hp, [128, ROWS] = sum_k wq[k][:,hp]^T @ xf[k]
            for hp in range(HP):
                for n0 in range(0, ROWS, 512):
                    ps = psA.tile([128, 512], dt.float32, tag="psA", name="psA")
                    for k in range(HP):
                        nc.tensor.matmul(
                            ps, wq_sb[k][:, hp * 128:(hp + 1) * 128],
                            xf_sb[k][:, n0:n0 + 512],
                            start=(k == 0), stop=(k == HP - 1))
                    nc.vector.tensor_scalar_add(
                        out=qt_sb[hp][:, n0:n0 + 512], in0=ps,
                        scalar1=bias_sb[:, hp:hp + 1])
            # KT
            for hp in range(HP):
                for n0 in range(0, ST, 512):
                    ps = psA.tile([128, 512], dt.float32, tag="psA", name="psA")
                    for k in range(HP):
                        nc.tensor.matmul(
                            ps, wk_sb[k][:, hp * 128:(hp + 1) * 128],
                            xt_sb[k][:, n0:n0 + 512],
                            start=(k == 0), stop=(k == HP - 1))
                    nc.vector.tensor_scalar_add(
                        out=kt_sb[hp][:, n0:n0 + 512], in0=ps,
                        scalar1=bias_sb[:, HP + hp:HP + hp + 1])
            # V (+bias row +ones cols): [128st, VW] = xt_aug[:, st]^T^T... lhsT=xt chunks
            for st in range(NST):
                c0 = st * 128
                for n0, nw in ((0, 512), (512, VW - 512)):
                    ps = psA.tile([128, 512], dt.float32, tag="psA", name="psA")
                    for k in range(HP):
                        nc.tensor.matmul(
                            ps[:, :nw], xt_sb[k][:, c0:c0 + 128],
                            wv_sb[k][:, n0:n0 + nw],
                            start=(k == 0), stop=False)
                    nc.tensor.matmul(
                        ps[:, :nw], ones_sb[:, c0:c0 + 128],
                        wv_bias[:, n0:n0 + nw],
                        start=False, stop=True)
                    nc.vector.tensor_copy(out=v_sb[st][:, n0:n0 + nw], in_=ps[:, :nw])

        # mask tiles loaded after projIn closes (reuses freed SBUF)
        mask_sb = []
        for st in range(NST):
            t = persist.tile([128, ROWS], dt.bfloat16, tag=f"mask{st}", name=f"mask{st}")
            nc.sync.dma_start(out=t, in_=maskT[st * 128:(st + 1) * 128, :])
            mask_sb.append(t)

        # ---------------- phase B: attention ----------------
        with tc.tile_pool(name="attn", bufs=3) as attn, \
             tc.tile_pool(name="rztmp", bufs=2) as rztmp, \
             tc.tile_pool(name="psS", bufs=1, space="PSUM") as psS, \
             tc.tile_pool(name="psC", bufs=1, space="PSUM") as psC:
            for hp in range(HP):
                ctxps = [psC.tile([HD + 1, ROWS], dt.float32, tag=f"ctxps{h2}", name=f"ctxps{h2}")
                         for h2 in range(2)]
                for st in range(NST):
                    c0 = st * 128
                    ptiles = []
                    for h2 in range(2):
                        off = HD * h2
                        sps = psS.tile([128, ROWS], dt.float32, tag=f"sps{h2}", name=f"sps{h2}")
                        for n0 in range(0, ROWS, 512):
                            nc.tensor.matmul(
                                sps[:, n0:n0 + 512],
                                kt_sb[hp][off:off + HD, c0:c0 + 128],
                                qt_sb[hp][off:off + HD, n0:n0 + 512],
                                start=True, stop=True)
                        p = attn.tile([128, ROWS], dt.bfloat16, tag="p", name="p")
                        nc.scalar.activation(out=p, in_=sps, func=EXP)
                        nc.vector.tensor_mul(out=p, in0=p, in1=mask_sb[st])
                        ptiles.append(p)
                    for h2 in range(2):
                        h = 2 * hp + h2
                        for n0 in range(0, ROWS, 512):
                            nc.tensor.matmul(
                                ctxps[h2][:, n0:n0 + 512],
                                v_sb[st][:, h * (HD + 1):(h + 1) * (HD + 1)],
                                ptiles[h2][:, n0:n0 + 512],
                                start=(st == 0), stop=(st == NST - 1))
                for h2 in range(2):
                    h = 2 * hp + h2
                    off = HD * h2
                    nc.vector.tensor_copy(out=ctx_sb[hp][off:off + HD, :],
                                          in_=ctxps[h2][0:HD, :])
                    rz = rztmp.tile([HD + 1, ROWS], dt.float32, tag="rz", name="rz")
                    nc.vector.reciprocal(out=rz[HD:HD + 1, :],
                                         in_=ctxps[h2][HD:HD + 1, :])
                    nc.sync.dma_start(out=rz_dram[h:h + 1, :],
                                      in_=rz[HD:HD + 1, :])

        # ---------------- phase C: normalize + output projection ----------------
        with tc.tile_pool(name="ctxn", bufs=HP) as ctxnp, \
             tc.tile_pool(name="rzbc", bufs=2) as rzbcp, \
             tc.tile_pool(name="outsb", bufs=2) as outsbp, \
             tc.tile_pool(name="psO", bufs=4, space="PSUM") as psO:
            ctxn = []
            for hp in range(HP):
                bc = rzbcp.tile([128, ROWS], dt.float32, tag="rzbc", name="rzbc")
                src = rz_dram[2 * hp:2 * hp + 2, :]
                bcast = bass.AP(tensor=src.tensor, offset=src.offset,
                                ap=[src.ap[0], [0, HD], src.ap[1]])
                nc.sync.dma_start(out=bc, in_=bcast)
                t = ctxnp.tile([128, ROWS], dt.bfloat16, tag=f"ctxn{hp}", name=f"ctxn{hp}")
                nc.vector.tensor_mul(out=t, in0=ctx_sb[hp], in1=bc)
                ctxn.append(t)
            for of in range(HP):
                o = outsbp.tile([128, ROWS], dt.float32, tag="outsb", name="outsb")
                for n0 in range(0, ROWS, 512):
                    ps = psO.tile([128, 512], dt.float32, tag="psO", name="psO")
                    for k in range(HP):
                        nc.tensor.matmul(
                            ps, wo_sb[k][:, of * 128:(of + 1) * 128],
                            ctxn[k][:, n0:n0 + 512],
                            start=(k == 0), stop=(k == HP - 1))
                    nc.vector.tensor_scalar_add(
                        out=o[:, n0:n0 + 512], in0=ps,
                        scalar1=bias_sb[:, 2 * HP + of:2 * HP + of + 1])
                nc.sync.dma_start(out=out[of * 128:(of + 1) * 128, :], in_=o)

        persist_cm.__exit__(None, None, None)

    nc.compile()
    return nc


def _get_nc():
    global _CACHED_NC
    if _CACHED_NC is None:
        _CACHED_NC = _build_nc()
    return _CACHED_NC


def _prep_inputs(from_tensor, to_tensor, attention_mask,
                 Wq, bq, Wk, bk, Wv, bv, Wo, bo):
    f32 = np.float32
    from_tensor = np.asarray(from_tensor, f32)
    to_tensor = np.asarray(to_tensor, f32)
    attention_mask = np.asarray(attention_mask)

    wq_h = (np.asarray(Wq, f32) * SCALE).astype(BF16)
    wk_h = np.asarray(Wk, f32).astype(BF16)
    wo_h = np.asarray(Wo, f32).astype(BF16)
    wv_aug = np.zeros((DIM + 1, VW), f32)
    Wv = np.asarray(Wv, f32)
    bv = np.asarray(bv, f32)
    for h in range(NH):
        wv_aug[:DIM, h * (HD + 1):h * (HD + 1) + HD] = Wv[:, h * HD:(h + 1) * HD]
        wv_aug[DIM, h * (HD + 1):h * (HD + 1) + HD] = bv[h * HD:(h + 1) * HD]
        wv_aug[DIM, h * (HD + 1) + HD] = 1.0
    wv_h = wv_aug.astype(BF16)

    biases = np.zeros((128, 3 * HP), f32)
    biases[:, 0:HP] = (np.asarray(bq, f32) * SCALE).reshape(HP, 128).T
    biases[:, HP:2 * HP] = np.asarray(bk, f32).reshape(HP, 128).T
    biases[:, 2 * HP:3 * HP] = np.asarray(bo, f32).reshape(HP, 128).T

    xtT_all = [np.ascontiguousarray(to_tensor[b].T).astype(BF16) for b in range(B)]

    in_maps = []
    for c in range(NCORES):
        b, half = c // 2, c % 2
        r0 = half * ROWS
        xfT = np.ascontiguousarray(from_tensor[b, r0:r0 + ROWS, :].T).astype(BF16)
        maskT = np.ascontiguousarray(
            attention_mask[b, r0:r0 + ROWS, :].T).astype(BF16)
        in_maps.append({
            "xfT": xfT, "xtT": xtT_all[b], "maskT": maskT,
            "wq": wq_h, "wk": wk_h, "wv": wv_h, "wo": wo_h, "biases": biases,
        })
    return in_maps


def _assemble(results):
    out = np.empty((B, SF, DIM), np.float32)
    for c, r in enumerate(results):
        b, half = c // 2, c % 2
        r0 = half * ROWS
        out[b, r0:r0 + ROWS, :] = np.asarray(r["out"], np.float32).T
    return out


def _run(in_maps, trace=False):
    from concourse.bass_utils import run_bass_kernel_spmd
    nc = _get_nc()
    return run_bass_kernel_spmd(nc, in_maps, core_ids=list(range(NCORES)),
                                trace=trace)


def kernel(**inputs):
    in_maps = _prep_inputs(**inputs)
    res = _run(in_maps, trace=False)
    return _assemble(res.results)


def kernel_profiled(**inputs):
    """Returns (output, exec_time_ns, trace_path)."""
    in_maps = _prep_inputs(**inputs)
    res = _run(in_maps, trace=True)
    trace_path = None
    if res.instructions_and_trace is not None:
        trace_path = res.instructions_and_trace[1]
    return _assemble(res.results), res.exec_time_ns, trace_path


# revision 6
# speedup vs baseline: 1.1430x; 1.1430x over previous
"""Multi-head attention on 8 TRN2 NeuronCores (Bass/Tile, SPMD, no collectives).

Problem: B=4, Sf=St=2048, DIM=768, H=12, Dh=64, f32 reference.

Sharding: (batch, Sf/2) -> 8 shards. Core c handles batch b=c//2, query rows
[512*(c%2)*2 : +1024). K/V projections for a batch are recomputed by both cores
of the pair (cheaper than any collective).

Device dataflow is fully transposed so no on-chip transposes are needed:
  QT[h]  [64,1024]  = Wq_h^T @ xf^T        (lhsT=Wq cols, rhs=xf^T)
  KT[h]  [64,2048]  = Wk_h^T @ xt^T
  V      [2048,780] = xt_aug^T^T @ Wv_aug  (per-head ones column folded in)
  S^T    [St,Sq]    = KT_chunk^T^T ... per (head, st-tile): lhsT=KT[64,128], rhs=QT[64,1024]
  P^T    = exp(S^T) * mask^T               (no max-subtract; masked lanes exp->*0)
  ctx^T/Z[65,1024]  = [V_h|1]^T @ P^T      (accumulated over 16 st-tiles in PSUM)
  out^T  [768,1024] = Wo^T @ (ctx^T * 1/Z) + bo
Host transposes out^T back and stitches the 8 shards.
"""

import os
import numpy as np
import ml_dtypes

BF16 = ml_dtypes.bfloat16

B, SF, ST, DIM = 4, 2048, 2048, 768
NH, HD = 12, 64
SCALE = HD ** -0.5
NCORES = 8
ROWS = B * SF // NCORES      # 1024 query rows per core
HP = NH // 2                 # 6 head-pairs == 6 x 128-partition chunks of DIM
VW = NH * (HD + 1)           # 780: V width with per-head ones columns
NST = ST // 128              # 16 st tiles

_CACHED_NC = None


def _build_nc():
    from concourse import bacc, tile, mybir
    import concourse.bass as bass

    dt = mybir.dt
    nc = bacc.Bacc("TRN2", target_bir_lowering=False, debug=False,
                   num_devices=NCORES)

    xfT = nc.dram_tensor("xfT", [DIM, ROWS], dt.bfloat16, kind="ExternalInput").ap()
    xtT = nc.dram_tensor("xtT", [DIM, ST], dt.bfloat16, kind="ExternalInput").ap()
    maskT = nc.dram_tensor("maskT", [ST, ROWS], dt.bfloat16, kind="ExternalInput").ap()
    wq = nc.dram_tensor("wq", [DIM, DIM], dt.bfloat16, kind="ExternalInput").ap()
    wk = nc.dram_tensor("wk", [DIM, DIM], dt.bfloat16, kind="ExternalInput").ap()
    wv = nc.dram_tensor("wv", [DIM + 1, VW], dt.bfloat16, kind="ExternalInput").ap()
    wo = nc.dram_tensor("wo", [DIM, DIM], dt.bfloat16, kind="ExternalInput").ap()
    biases = nc.dram_tensor("biases", [128, 3 * HP], dt.float32, kind="ExternalInput").ap()
    out = nc.dram_tensor("out", [DIM, ROWS], dt.float32, kind="ExternalOutput").ap()
    rz_dram = nc.dram_tensor("rz_scratch", [NH, ROWS], dt.float32).ap()

    EXP = mybir.ActivationFunctionType.Exp

    with tile.TileContext(nc) as tc:
        persist_cm = tc.tile_pool(name="persist", bufs=1)
        persist = persist_cm.__enter__()

        wo_sb = []
        for k in range(HP):
            t = persist.tile([128, DIM], dt.bfloat16, tag=f"wo{k}", name=f"wo{k}")
            nc.sync.dma_start(out=t, in_=wo[k * 128:(k + 1) * 128, :])
            wo_sb.append(t)
        bias_sb = persist.tile([128, 3 * HP], dt.float32, tag="biases", name="biases")
        nc.sync.dma_start(out=bias_sb, in_=biases)

        qt_sb = [persist.tile([128, ROWS], dt.bfloat16, tag=f"qt{i}", name=f"qt{i}") for i in range(HP)]
        kt_sb = [persist.tile([128, ST], dt.bfloat16, tag=f"kt{i}", name=f"kt{i}") for i in range(HP)]
        v_sb = [persist.tile([128, VW], dt.bfloat16, tag=f"v{i}", name=f"v{i}") for i in range(NST)]
        ctx_sb = [persist.tile([128, ROWS], dt.bfloat16, tag=f"ctx{i}", name=f"ctx{i}") for i in range(HP)]

        # ---------------- phase A: projections ----------------
        with tc.tile_pool(name="projIn", bufs=1) as projin, \
             tc.tile_pool(name="psA", bufs=4, space="PSUM") as psA:
            xf_sb, xt_sb, wq_sb, wk_sb, wv_sb = [], [], [], [], []
            for k in range(HP):
                t = projin.tile([128, ROWS], dt.bfloat16, tag=f"xf{k}", name=f"xf{k}")
                nc.sync.dma_start(out=t, in_=xfT[k * 128:(k + 1) * 128, :])
                xf_sb.append(t)
                t = projin.tile([128, DIM], dt.bfloat16, tag=f"wq{k}", name=f"wq{k}")
                nc.sync.dma_start(out=t, in_=wq[k * 128:(k + 1) * 128, :])
                wq_sb.append(t)
            for k in range(HP):
                t = projin.tile([128, ST], dt.bfloat16, tag=f"xt{k}", name=f"xt{k}")
                nc.sync.dma_start(out=t, in_=xtT[k * 128:(k + 1) * 128, :])
                xt_sb.append(t)
                t = projin.tile([128, DIM], dt.bfloat16, tag=f"wk{k}", name=f"wk{k}")
                nc.sync.dma_start(out=t, in_=wk[k * 128:(k + 1) * 128, :])
                wk_sb.append(t)
                t = projin.tile([128, VW], dt.bfloat16, tag=f"wv{k}", name=f"wv{k}")
                nc.sync.dma_start(out=t, in_=wv[k * 128:(k + 1) * 128, :])
                wv_sb.append(t)
            wv_bias = projin.tile([1, VW], dt.bfloat16, tag="wvb", name="wvb")
            nc.sync.dma_start(out=wv_bias, in_=wv[DIM:DIM + 1, :])
            ones_sb = projin.tile([1, ST], dt.bfloat16, tag="ones", name="ones")
            nc.vector.memset(ones_sb, 1.0)

            # QT: per head-pair hp, [128, ROWS] = sum_k wq[k][:,hp]^T @ xf[k]
            for hp in range(HP):
                for n0 in range(0, ROWS, 512):
                    ps = psA.tile([128, 512], dt.float32, tag="psA", name="psA")
                    for k in range(HP):
                        nc.tensor.matmul(
                            ps, wq_sb[k][:, hp * 128:(hp + 1) * 128],
                            xf_sb[k][:, n0:n0 + 512],
                            start=(k == 0), stop=(k == HP - 1))
                    nc.vector.tensor_scalar_add(
                        out=qt_sb[hp][:, n0:n0 + 512], in0=ps,
                        scalar1=bias_sb[:, hp:hp + 1])
            # KT
            for hp in range(HP):
                for n0 in range(0, ST, 512):
                    ps = psA.tile([128, 512], dt.float32, tag="psA", name="psA")
                    for k in range(HP):
                        nc.tensor.matmul(
                            ps, wk_sb[k][:, hp * 128:(hp + 1) * 128],
                            xt_sb[k][:, n0:n0 + 512],
                            start=(k == 0), stop=(k == HP - 1))
                    nc.vector.tensor_scalar_add(
                        out=kt_sb[hp][:, n0:n0 + 512], in0=ps,
                        scalar1=bias_sb[:, HP + hp:HP + hp + 1])
            # V (+bias row +ones cols): [128st, VW] = xt_aug[:, st]^T^T... lhsT=xt chunks
            for st in range(NST):
                c0 = st * 128
                for n0, nw in ((0, 512), (512, VW - 512)):
                    ps = psA.tile([128, 512], dt.float32, tag="psA", name="psA")
                    for k in range(HP):
                        nc.tensor.matmul(
                            ps[:, :nw], xt_sb[k][:, c0:c0 + 128],
                            wv_sb[k][:, n0:n0 + nw],
                            start=(k == 0), stop=False)
                    nc.tensor.matmul(
                        ps[:, :nw], ones_sb[:, c0:c0 + 128],
                        wv_bias[:, n0:n0 + nw],
                        start=False, stop=True)
                    nc.vector.tensor_copy(out=v_sb[st][:, n0:n0 + nw], in_=ps[:, :nw])

        # mask tiles loaded after projIn closes (reuses freed SBUF)
        mask_sb = []
        for st in range(NST):
            t = persist.tile([128, ROWS], dt.bfloat16, tag=f"mask{st}", name=f"mask{st}")
            nc.sync.dma_start(out=t, in_=maskT[st * 128:(st + 1) * 128, :])
            mask_sb.append(t)

        # ---------------- phase B: attention ----------------
        # Software-pipelined emission: ctx matmuls for step k are emitted after
        # the scores matmuls for step k+1, so the in-order PE stream never
        # stalls long enough on the exp->mask chain to let HAM re-throttle.
        ctxn = [persist.tile([128, ROWS], dt.bfloat16, tag=f"ctxn{i}", name=f"ctxn{i}")
                for i in range(HP)]
        with tc.tile_pool(name="attn", bufs=3) as attn, \
             tc.tile_pool(name="zrow", bufs=2) as zrowp, \
             tc.tile_pool(name="z2", bufs=2) as z2p, \
             tc.tile_pool(name="rzbc", bufs=2) as rzbcp, \
             tc.tile_pool(name="psS", bufs=2, space="PSUM") as psS, \
             tc.tile_pool(name="psC", bufs=2, space="PSUM") as psC:

            steps = [(h, st) for h in range(NH) for st in range(NST)]
            pending = None  # (h, st, ptile, ctxps)
            ctxps_cur = None
            z2_cur = None

            def emit_ctx(h, st, p, cps):
                for n0 in range(0, ROWS, 512):
                    nc.tensor.matmul(
                        cps[:, n0:n0 + 512],
                        v_sb[st][:, h * (HD + 1):(h + 1) * (HD + 1)],
                        p[:, n0:n0 + 512],
                        start=(st == 0), stop=(st == NST - 1))

            def drain_head(h, cps):
                nonlocal z2_cur
                hp, h2 = h // 2, h % 2
                off = HD * h2
                nc.vector.tensor_copy(out=ctx_sb[hp][off:off + HD, :],
                                      in_=cps[0:HD, :])
                if h2 == 0:
                    z2_cur = z2p.tile([2, ROWS], dt.float32, tag="z2", name="z2")
                zrow = zrowp.tile([HD + 1, ROWS], dt.float32, tag="zrow", name="zrow")
                nc.vector.tensor_copy(out=zrow[HD:HD + 1, :],
                                      in_=cps[HD:HD + 1, :])
                nc.sync.dma_start(out=z2_cur[h2:h2 + 1, :],
                                  in_=zrow[HD:HD + 1, :])
                if h2 == 1:
                    rz2 = z2p.tile([2, ROWS], dt.float32, tag="rz2", name="rz2")
                    nc.vector.reciprocal(out=rz2, in_=z2_cur)
                    nc.sync.dma_start(out=rz_dram[2 * hp:2 * hp + 2, :], in_=rz2)
                    bc = rzbcp.tile([128, ROWS], dt.float32, tag="rzbc", name="rzbc")
                    src = rz_dram[2 * hp:2 * hp + 2, :]
                    bcast = bass.AP(tensor=src.tensor, offset=src.offset,
                                    ap=[src.ap[0], [0, HD], src.ap[1]])
                    nc.sync.dma_start(out=bc, in_=bcast)
                    nc.vector.tensor_mul(out=ctxn[hp], in0=ctx_sb[hp], in1=bc)

            for (h, st) in steps:
                hp, h2 = h // 2, h % 2
                off = HD * h2
                c0 = st * 128
                if st == 0:
                    ctxps_cur = psC.tile([HD + 1, ROWS], dt.float32,
                                         tag="ctxps", name="ctxps")
                sps = psS.tile([128, ROWS], dt.float32, tag="sps", name="sps")
                for n0 in range(0, ROWS, 512):
                    nc.tensor.matmul(
                        sps[:, n0:n0 + 512],
                        kt_sb[hp][off:off + HD, c0:c0 + 128],
                        qt_sb[hp][off:off + HD, n0:n0 + 512],
                        start=True, stop=True)
                p = attn.tile([128, ROWS], dt.bfloat16, tag="p", name="p")
                nc.scalar.activation(out=p, in_=sps, func=EXP)
                nc.vector.tensor_mul(out=p, in0=p, in1=mask_sb[st])
                if pending is not None:
                    ph, pst, pp, pcps = pending
                    emit_ctx(ph, pst, pp, pcps)
                    if pst == NST - 1:
                        drain_head(ph, pcps)
                pending = (h, st, p, ctxps_cur)
            ph, pst, pp, pcps = pending
            emit_ctx(ph, pst, pp, pcps)
            drain_head(ph, pcps)

        # ---------------- phase C: output projection ----------------
        with tc.tile_pool(name="outsb", bufs=2) as outsbp, \
             tc.tile_pool(name="psO", bufs=4, space="PSUM") as psO:
            for of in range(HP):
                o = outsbp.tile([128, ROWS], dt.float32, tag="outsb", name="outsb")
                for n0 in range(0, ROWS, 512):
                    ps = psO.tile([128, 512], dt.float32, tag="psO", name="psO")
                    for k in range(HP):
                        nc.tensor.matmul(
                            ps, wo_sb[k][:, of * 128:(of + 1) * 128],
                            ctxn[k][:, n0:n0 + 512],
                            start=(k == 0), stop=(k == HP - 1))
                    nc.vector.tensor_scalar_add(
                        out=o[:, n0:n0 + 512], in0=ps,
                        scalar1=bias_sb[:, 2 * HP + of:2 * HP + of + 1])
                nc.sync.dma_start(out=out[of * 128:(of + 1) * 128, :], in_=o)

        persist_cm.__exit__(None, None, None)

    nc.compile()
    return nc


def _get_nc():
    global _CACHED_NC
    if _CACHED_NC is None:
        _CACHED_NC = _build_nc()
    return _CACHED_NC


def _prep_inputs(from_tensor, to_tensor, attention_mask,
                 Wq, bq, Wk, bk, Wv, bv, Wo, bo):
    f32 = np.float32
    from_tensor = np.asarray(from_tensor, f32)
    to_tensor = np.asarray(to_tensor, f32)
    attention_mask = np.asarray(attention_mask)

    wq_h = (np.asarray(Wq, f32) * SCALE).astype(BF16)
    wk_h = np.asarray(Wk, f32).astype(BF16)
    wo_h = np.asarray(Wo, f32).astype(BF16)
    wv_aug = np.zeros((DIM + 1, VW), f32)
    Wv = np.asarray(Wv, f32)
    bv = np.asarray(bv, f32)
    for h in range(NH):
        wv_aug[:DIM, h * (HD + 1):h * (HD + 1) + HD] = Wv[:, h * HD:(h + 1) * HD]
        wv_aug[DIM, h * (HD + 1):h * (HD + 1) + HD] = bv[h * HD:(h + 1) * HD]
        wv_aug[DIM, h * (HD + 1) + HD] = 1.0
    wv_h = wv_aug.astype(BF16)

    biases = np.zeros((128, 3 * HP), f32)
    biases[:, 0:HP] = (np.asarray(bq, f32) * SCALE).reshape(HP, 128).T
    biases[:, HP:2 * HP] = np.asarray(bk, f32).reshape(HP, 128).T
    biases[:, 2 * HP:3 * HP] = np.asarray(bo, f32).reshape(HP, 128).T

    xtT_all = [np.ascontiguousarray(to_tensor[b].T).astype(BF16) for b in range(B)]

    in_maps = []
    for c in range(NCORES):
        b, half = c // 2, c % 2
        r0 = half * ROWS
        xfT = np.ascontiguousarray(from_tensor[b, r0:r0 + ROWS, :].T).astype(BF16)
        maskT = np.ascontiguousarray(
            attention_mask[b, r0:r0 + ROWS, :].T).astype(BF16)
        in_maps.append({
            "xfT": xfT, "xtT": xtT_all[b], "maskT": maskT,
            "wq": wq_h, "wk": wk_h, "wv": wv_h, "wo": wo_h, "biases": biases,
        })
    return in_maps


def _assemble(results):
    out = np.empty((B, SF, DIM), np.float32)
    for c, r in enumerate(results):
        b, half = c // 2, c % 2
        r0 = half * ROWS
        out[b, r0:r0 + ROWS, :] = np.asarray(r["out"], np.float32).T
    return out


def _run(in_maps, trace=False):
    from concourse.bass_utils import run_bass_kernel_spmd
    nc = _get_nc()
    return run_bass_kernel_spmd(nc, in_maps, core_ids=list(range(NCORES)),
                                trace=trace)


def kernel(**inputs):
    in_maps = _prep_inputs(**inputs)
    res = _run(in_maps, trace=False)
    return _assemble(res.results)


def kernel_profiled(**inputs):
    """Returns (output, exec_time_ns, trace_path)."""
    in_maps = _prep_inputs(**inputs)
    res = _run(in_maps, trace=True)
    trace_path = None
    if res.instructions_and_trace is not None:
        trace_path = res.instructions_and_trace[1]
    return _assemble(res.results), res.exec_time_ns, trace_path


# revision 8
# speedup vs baseline: 1.1892x; 1.0404x over previous
"""Multi-head attention on 8 TRN2 NeuronCores (Bass/Tile, SPMD, no collectives).

Problem: B=4, Sf=St=2048, DIM=768, H=12, Dh=64, f32 reference.

Sharding: (batch, Sf/2) -> 8 shards. Core c handles batch b=c//2, query rows
[512*(c%2)*2 : +1024). K/V projections for a batch are recomputed by both cores
of the pair (cheaper than any collective).

Device dataflow is fully transposed so no on-chip transposes are needed:
  QT[h]  [64,1024]  = Wq_h^T @ xf^T        (lhsT=Wq cols, rhs=xf^T)
  KT[h]  [64,2048]  = Wk_h^T @ xt^T
  V      [2048,768] = xt_aug^T @ Wv_aug   (bias row via K=1 matmul)
  S^T    [St,Sq]    = KT_chunk^T^T ... per (head, st-tile): lhsT=KT[64,128], rhs=QT[64,1024]
  P^T    = exp(S^T) * mask^T               (no max-subtract; masked lanes exp->*0)
  ctx^T  [128,1024] = V^T @ P^T, two heads col-packed via tile_position;
  Z      via 4x col-packed M=1 ones-matmuls  (both accumulated over st in PSUM)
  out^T  [768,1024] = Wo^T @ (ctx^T * 1/Z) + bo
Host transposes out^T back and stitches the 8 shards.
"""

import os
import numpy as np
import ml_dtypes

BF16 = ml_dtypes.bfloat16

B, SF, ST, DIM = 4, 2048, 2048, 768
NH, HD = 12, 64
SCALE = HD ** -0.5
NCORES = 8
ROWS = B * SF // NCORES      # 1024 query rows per core
HP = NH // 2                 # 6 head-pairs == 6 x 128-partition chunks of DIM
VW = DIM                     # 768: V width (no ones columns; Z via packed M=1 matmuls)
NST = ST // 128              # 16 st tiles

_CACHED_NC = None


def _build_nc():
    from concourse import bacc, tile, mybir
    import concourse.bass as bass

    dt = mybir.dt
    nc = bacc.Bacc("TRN2", target_bir_lowering=False, debug=False,
                   num_devices=NCORES)

    xfT = nc.dram_tensor("xfT", [DIM, ROWS], dt.bfloat16, kind="ExternalInput").ap()
    xtT = nc.dram_tensor("xtT", [DIM, ST], dt.bfloat16, kind="ExternalInput").ap()
    maskT = nc.dram_tensor("maskT", [ST, ROWS], dt.bfloat16, kind="ExternalInput").ap()
    wq = nc.dram_tensor("wq", [DIM, DIM], dt.bfloat16, kind="ExternalInput").ap()
    wk = nc.dram_tensor("wk", [DIM, DIM], dt.bfloat16, kind="ExternalInput").ap()
    wv = nc.dram_tensor("wv", [DIM + 1, VW], dt.bfloat16, kind="ExternalInput").ap()
    wo = nc.dram_tensor("wo", [DIM, DIM], dt.bfloat16, kind="ExternalInput").ap()
    biases = nc.dram_tensor("biases", [128, 3 * HP], dt.float32, kind="ExternalInput").ap()
    out = nc.dram_tensor("out", [DIM, ROWS], dt.float32, kind="ExternalOutput").ap()
    rz_dram = nc.dram_tensor("rz_scratch", [NH, ROWS], dt.float32).ap()

    EXP = mybir.ActivationFunctionType.Exp

    with tile.TileContext(nc) as tc:
        persist_cm = tc.tile_pool(name="persist", bufs=1)
        persist = persist_cm.__enter__()

        wo_sb = []
        for k in range(HP):
            t = persist.tile([128, DIM], dt.bfloat16, tag=f"wo{k}", name=f"wo{k}")
            nc.sync.dma_start(out=t, in_=wo[k * 128:(k + 1) * 128, :])
            wo_sb.append(t)
        bias_sb = persist.tile([128, 3 * HP], dt.float32, tag="biases", name="biases")
        nc.sync.dma_start(out=bias_sb, in_=biases)

        qt_sb = [persist.tile([128, ROWS], dt.bfloat16, tag=f"qt{i}", name=f"qt{i}") for i in range(HP)]
        kt_sb = [persist.tile([128, ST], dt.bfloat16, tag=f"kt{i}", name=f"kt{i}") for i in range(HP)]
        v_sb = [persist.tile([128, VW], dt.bfloat16, tag=f"v{i}", name=f"v{i}") for i in range(NST)]
        ctx_sb = [persist.tile([128, ROWS], dt.bfloat16, tag=f"ctx{i}", name=f"ctx{i}") for i in range(HP)]

        # ---------------- phase A: projections ----------------
        with tc.tile_pool(name="projIn", bufs=1) as projin, \
             tc.tile_pool(name="psA", bufs=4, space="PSUM") as psA:
            xf_sb, xt_sb, wq_sb, wk_sb, wv_sb = [], [], [], [], []
            for k in range(HP):
                t = projin.tile([128, ROWS], dt.bfloat16, tag=f"xf{k}", name=f"xf{k}")
                nc.sync.dma_start(out=t, in_=xfT[k * 128:(k + 1) * 128, :])
                xf_sb.append(t)
                t = projin.tile([128, DIM], dt.bfloat16, tag=f"wq{k}", name=f"wq{k}")
                nc.sync.dma_start(out=t, in_=wq[k * 128:(k + 1) * 128, :])
                wq_sb.append(t)
            for k in range(HP):
                t = projin.tile([128, ST], dt.bfloat16, tag=f"xt{k}", name=f"xt{k}")
                nc.sync.dma_start(out=t, in_=xtT[k * 128:(k + 1) * 128, :])
                xt_sb.append(t)
                t = projin.tile([128, DIM], dt.bfloat16, tag=f"wk{k}", name=f"wk{k}")
                nc.sync.dma_start(out=t, in_=wk[k * 128:(k + 1) * 128, :])
                wk_sb.append(t)
                t = projin.tile([128, VW], dt.bfloat16, tag=f"wv{k}", name=f"wv{k}")
                nc.sync.dma_start(out=t, in_=wv[k * 128:(k + 1) * 128, :])
                wv_sb.append(t)
            wv_bias = projin.tile([1, VW], dt.bfloat16, tag="wvb", name="wvb")
            nc.sync.dma_start(out=wv_bias, in_=wv[DIM:DIM + 1, :])
            ones_sb = projin.tile([1, ST], dt.bfloat16, tag="ones", name="ones")
            nc.vector.memset(ones_sb, 1.0)

            # QT: per head-pair hp, [128, ROWS] = sum_k wq[k][:,hp]^T @ xf[k]
            for hp in range(HP):
                for n0 in range(0, ROWS, 512):
                    ps = psA.tile([128, 512], dt.float32, tag="psA", name="psA")
                    for k in range(HP):
                        nc.tensor.matmul(
                            ps, wq_sb[k][:, hp * 128:(hp + 1) * 128],
                            xf_sb[k][:, n0:n0 + 512],
                            start=(k == 0), stop=(k == HP - 1))
                    nc.vector.tensor_scalar_add(
                        out=qt_sb[hp][:, n0:n0 + 512], in0=ps,
                        scalar1=bias_sb[:, hp:hp + 1])
            # KT
            for hp in range(HP):
                for n0 in range(0, ST, 512):
                    ps = psA.tile([128, 512], dt.float32, tag="psA", name="psA")
                    for k in range(HP):
                        nc.tensor.matmul(
                            ps, wk_sb[k][:, hp * 128:(hp + 1) * 128],
                            xt_sb[k][:, n0:n0 + 512],
                            start=(k == 0), stop=(k == HP - 1))
                    nc.vector.tensor_scalar_add(
                        out=kt_sb[hp][:, n0:n0 + 512], in0=ps,
                        scalar1=bias_sb[:, HP + hp:HP + hp + 1])
            # V (+bias row +ones cols): [128st, VW] = xt_aug[:, st]^T^T... lhsT=xt chunks
            for st in range(NST):
                c0 = st * 128
                for n0, nw in ((0, 512), (512, VW - 512)):
                    ps = psA.tile([128, 512], dt.float32, tag="psA", name="psA")
                    for k in range(HP):
                        nc.tensor.matmul(
                            ps[:, :nw], xt_sb[k][:, c0:c0 + 128],
                            wv_sb[k][:, n0:n0 + nw],
                            start=(k == 0), stop=False)
                    nc.tensor.matmul(
                        ps[:, :nw], ones_sb[:, c0:c0 + 128],
                        wv_bias[:, n0:n0 + nw],
                        start=False, stop=True)
                    nc.vector.tensor_copy(out=v_sb[st][:, n0:n0 + nw], in_=ps[:, :nw])

        # mask tiles loaded after projIn closes (reuses freed SBUF)
        mask_sb = []
        for st in range(NST):
            t = persist.tile([128, ROWS], dt.bfloat16, tag=f"mask{st}", name=f"mask{st}")
            nc.sync.dma_start(out=t, in_=maskT[st * 128:(st + 1) * 128, :])
            mask_sb.append(t)

        # ---------------- phase B: attention ----------------
        # Head-PAIR processing with explicit tile_position packing:
        #  - scores: the two heads' K=64 matmuls row-packed at (0,0)/(64,0)
        #  - ctx:    the two heads' M=64 matmuls col-packed at (0,0)/(0,64)
        #            into ONE [128, ROWS] psum tile (head h rows 0:64, h' 64:128)
        #  - Z:      4x M=1 ones-matmuls col-packed at (0,32j)
        # Emission is software-pipelined (ctx for step k after scores for k+1)
        # so the in-order PE stream doesn't stall on the exp->mask chain.
        ones_col = persist.tile([128, 1], dt.bfloat16, tag="ones_col", name="ones_col")
        nc.vector.memset(ones_col, 1.0)
        ctxn = [persist.tile([128, ROWS], dt.bfloat16, tag=f"ctxn{i}", name=f"ctxn{i}")
                for i in range(HP)]
        ZJ = [(0, 0), (0, 512), (1, 0), (1, 512)]  # (h2, n0) per zps row 32j
        with tc.tile_pool(name="attn", bufs=6) as attn, \
             tc.tile_pool(name="z97", bufs=2) as z97p, \
             tc.tile_pool(name="z2", bufs=2) as z2p, \
             tc.tile_pool(name="rzbc", bufs=2) as rzbcp, \
             tc.tile_pool(name="psS", bufs=2, space="PSUM") as psS, \
             tc.tile_pool(name="psC", bufs=1, space="PSUM") as psC, \
             tc.tile_pool(name="psZ", bufs=1, space="PSUM") as psZ:

            pending = None
            ctxp_cur = None
            zps_cur = None

            def emit_ctx(hp, st, pp, ctxp, zps):
                for n0 in range(0, ROWS, 512):
                    for h2 in range(2):
                        nc.tensor.matmul(
                            ctxp[64 * h2:64 * h2 + 64, n0:n0 + 512],
                            v_sb[st][:, (2 * hp + h2) * HD:(2 * hp + h2 + 1) * HD],
                            pp[h2][:, n0:n0 + 512],
                            start=(st == 0), stop=(st == NST - 1),
                            tile_position=(0, 64 * h2))
                for j, (h2, n0) in enumerate(ZJ):
                    nc.tensor.matmul(
                        zps[32 * j:32 * j + 1, 0:512],
                        ones_col,
                        pp[h2][:, n0:n0 + 512],
                        start=(st == 0), stop=(st == NST - 1),
                        tile_position=(0, 32 * j))

            def drain_pair(hp, ctxp, zps):
                nc.vector.tensor_copy(out=ctx_sb[hp], in_=ctxp)
                z97 = z97p.tile([97, 512], dt.float32, tag="z97", name="z97")
                nc.vector.tensor_copy(out=z97, in_=zps[0:97, 0:512])
                z2 = z2p.tile([2, ROWS], dt.float32, tag="z2", name="z2")
                for j, (h2, n0) in enumerate(ZJ):
                    nc.sync.dma_start(out=z2[h2:h2 + 1, n0:n0 + 512],
                                      in_=z97[32 * j:32 * j + 1, :])
                rz2 = z2p.tile([2, ROWS], dt.float32, tag="rz2", name="rz2")
                nc.vector.reciprocal(out=rz2, in_=z2)
                nc.sync.dma_start(out=rz_dram[2 * hp:2 * hp + 2, :], in_=rz2)
                bc = rzbcp.tile([128, ROWS], dt.float32, tag="rzbc", name="rzbc")
                src = rz_dram[2 * hp:2 * hp + 2, :]
                bcast = bass.AP(tensor=src.tensor, offset=src.offset,
                                ap=[src.ap[0], [0, HD], src.ap[1]])
                nc.sync.dma_start(out=bc, in_=bcast)
                nc.vector.tensor_mul(out=ctxn[hp], in0=ctx_sb[hp], in1=bc)

            for hp in range(HP):
                for st in range(NST):
                    c0 = st * 128
                    if st == 0:
                        ctxp_cur = psC.tile([128, ROWS], dt.float32,
                                            tag="ctxp", name="ctxp")
                        zps_cur = psZ.tile([128, 512], dt.float32,
                                           tag="zps", name="zps")
                    sh = [psS.tile([128, ROWS], dt.float32, tag="sps", name="sps")
                          for _ in range(2)]
                    for n0 in range(0, ROWS, 512):
                        for h2 in range(2):
                            nc.tensor.matmul(
                                sh[h2][:, n0:n0 + 512],
                                kt_sb[hp][HD * h2:HD * h2 + HD, c0:c0 + 128],
                                qt_sb[hp][HD * h2:HD * h2 + HD, n0:n0 + 512],
                                start=True, stop=True,
                                tile_position=(64 * h2, 0))
                    pp = []
                    for h2 in range(2):
                        p = attn.tile([128, ROWS], dt.bfloat16, tag="p", name="p")
                        nc.scalar.activation(out=p, in_=sh[h2], func=EXP)
                        nc.vector.tensor_mul(out=p, in0=p, in1=mask_sb[st])
                        pp.append(p)
                    if pending is not None:
                        php, pst, ppp, pctxp, pzps = pending
                        emit_ctx(php, pst, ppp, pctxp, pzps)
                        if pst == NST - 1:
                            drain_pair(php, pctxp, pzps)
                    pending = (hp, st, pp, ctxp_cur, zps_cur)
            php, pst, ppp, pctxp, pzps = pending
            emit_ctx(php, pst, ppp, pctxp, pzps)
            drain_pair(php, pctxp, pzps)

        # ---------------- phase C: output projection ----------------
        with tc.tile_pool(name="outsb", bufs=2) as outsbp, \
             tc.tile_pool(name="psO", bufs=4, space="PSUM") as psO:
            for of in range(HP):
                o = outsbp.tile([128, ROWS], dt.float32, tag="outsb", name="outsb")
                for n0 in range(0, ROWS, 512):
                    ps = psO.tile([128, 512], dt.float32, tag="psO", name="psO")
                    for k in range(HP):
                        nc.tensor.matmul(
                            ps, wo_sb[k][:, of * 128:(of + 1) * 128],
                            ctxn[k][:, n0:n0 + 512],
                            start=(k == 0), stop=(k == HP - 1))
                    nc.vector.tensor_scalar_add(
                        out=o[:, n0:n0 + 512], in0=ps,
                        scalar1=bias_sb[:, 2 * HP + of:2 * HP + of + 1])
                nc.sync.dma_start(out=out[of * 128:(of + 1) * 128, :], in_=o)

        persist_cm.__exit__(None, None, None)

    nc.compile()
    return nc


def _get_nc():
    global _CACHED_NC
    if _CACHED_NC is None:
        _CACHED_NC = _build_nc()
    return _CACHED_NC


def _prep_inputs(from_tensor, to_tensor, attention_mask,
                 Wq, bq, Wk, bk, Wv, bv, Wo, bo):
    f32 = np.float32
    from_tensor = np.asarray(from_tensor, f32)
    to_tensor = np.asarray(to_tensor, f32)
    attention_mask = np.asarray(attention_mask)

    wq_h = (np.asarray(Wq, f32) * SCALE).astype(BF16)
    wk_h = np.asarray(Wk, f32).astype(BF16)
    wo_h = np.asarray(Wo, f32).astype(BF16)
    wv_aug = np.vstack([np.asarray(Wv, f32), np.asarray(bv, f32)[None, :]])
    wv_h = wv_aug.astype(BF16)

    biases = np.zeros((128, 3 * HP), f32)
    biases[:, 0:HP] = (np.asarray(bq, f32) * SCALE).reshape(HP, 128).T
    biases[:, HP:2 * HP] = np.asarray(bk, f32).reshape(HP, 128).T
    biases[:, 2 * HP:3 * HP] = np.asarray(bo, f32).reshape(HP, 128).T

    xtT_all = [np.ascontiguousarray(to_tensor[b].T).astype(BF16) for b in range(B)]

    in_maps = []
    for c in range(NCORES):
        b, half = c // 2, c % 2
        r0 = half * ROWS
        xfT = np.ascontiguousarray(from_tensor[b, r0:r0 + ROWS, :].T).astype(BF16)
        maskT = np.ascontiguousarray(
            attention_mask[b, r0:r0 + ROWS, :].T).astype(BF16)
        in_maps.append({
            "xfT": xfT, "xtT": xtT_all[b], "maskT": maskT,
            "wq": wq_h, "wk": wk_h, "wv": wv_h, "wo": wo_h, "biases": biases,
        })
    return in_maps


def _assemble(results):
    out = np.empty((B, SF, DIM), np.float32)
    for c, r in enumerate(results):
        b, half = c // 2, c % 2
        r0 = half * ROWS
        out[b, r0:r0 + ROWS, :] = np.asarray(r["out"], np.float32).T
    return out


def _run(in_maps, trace=False):
    from concourse.bass_utils import run_bass_kernel_spmd
    nc = _get_nc()
    return run_bass_kernel_spmd(nc, in_maps, core_ids=list(range(NCORES)),
                                trace=trace)


def kernel(**inputs):
    in_maps = _prep_inputs(**inputs)
    res = _run(in_maps, trace=False)
    return _assemble(res.results)


def kernel_profiled(**inputs):
    """Returns (output, exec_time_ns, trace_path)."""
    in_maps = _prep_inputs(**inputs)
    res = _run(in_maps, trace=True)
    trace_path = None
    if res.instructions_and_trace is not None:
        trace_path = res.instructions_and_trace[1]
    return _assemble(res.results), res.exec_time_ns, trace_path


# revision 9
# speedup vs baseline: 1.3654x; 1.1483x over previous
"""Multi-head attention on 8 TRN2 NeuronCores (Bass/Tile, SPMD, no collectives).

Problem: B=4, Sf=St=2048, DIM=768, H=12, Dh=64, f32 reference.

Sharding: (batch, Sf/2) -> 8 shards. Core c handles batch b=c//2, query rows
[512*(c%2)*2 : +1024). K/V projections for a batch are recomputed by both cores
of the pair (cheaper than any collective).

Device dataflow is fully transposed so no on-chip transposes are needed:
  QT[h]  [64,1024]  = Wq_h^T @ xf^T        (lhsT=Wq cols, rhs=xf^T)
  KT[h]  [64,2048]  = Wk_h^T @ xt^T
  V      [2048,768] = xt_aug^T @ Wv_aug   (bias row via K=1 matmul)
  S^T    [St,Sq]    = KT_chunk^T^T ... per (head, st-tile): lhsT=KT[64,128], rhs=QT[64,1024]
  P^T    = exp(S^T) * mask^T               (no max-subtract; masked lanes exp->*0)
  ctx^T  [128,1024] = V^T @ P^T, two heads col-packed via tile_position;
  Z      via 4x col-packed M=1 ones-matmuls  (both accumulated over st in PSUM)
  out^T  [768,1024] = Wo^T @ (ctx^T * 1/Z) + bo
Host transposes out^T back and stitches the 8 shards.
"""

import os
import numpy as np
import ml_dtypes

BF16 = ml_dtypes.bfloat16

B, SF, ST, DIM = 4, 2048, 2048, 768
NH, HD = 12, 64
SCALE = HD ** -0.5
NCORES = 8
ROWS = B * SF // NCORES      # 1024 query rows per core
HP = NH // 2                 # 6 head-pairs == 6 x 128-partition chunks of DIM
VW = DIM                     # 768: V width (no ones columns; Z via packed M=1 matmuls)
NST = ST // 128              # 16 st tiles

_CACHED_NC = None


def _build_nc():
    from concourse import bacc, tile, mybir
    import concourse.bass as bass

    dt = mybir.dt
    nc = bacc.Bacc("TRN2", target_bir_lowering=False, debug=False,
                   num_devices=NCORES)

    xfT = nc.dram_tensor("xfT", [DIM, ROWS], dt.bfloat16, kind="ExternalInput").ap()
    xtT = nc.dram_tensor("xtT", [DIM, ST], dt.bfloat16, kind="ExternalInput").ap()
    maskT = nc.dram_tensor("maskT", [ST, ROWS], dt.bfloat16, kind="ExternalInput").ap()
    wq = nc.dram_tensor("wq", [DIM, DIM], dt.bfloat16, kind="ExternalInput").ap()
    wk = nc.dram_tensor("wk", [DIM, DIM], dt.bfloat16, kind="ExternalInput").ap()
    wv = nc.dram_tensor("wv", [DIM + 1, VW], dt.bfloat16, kind="ExternalInput").ap()
    wo = nc.dram_tensor("wo", [DIM, DIM], dt.bfloat16, kind="ExternalInput").ap()
    biases = nc.dram_tensor("biases", [128, 3 * HP], dt.float32, kind="ExternalInput").ap()
    out = nc.dram_tensor("out", [DIM, ROWS], dt.float32, kind="ExternalOutput").ap()
    rz_dram = nc.dram_tensor("rz_scratch", [NH, ROWS], dt.bfloat16).ap()

    EXP = mybir.ActivationFunctionType.Exp

    with tile.TileContext(nc) as tc:
        persist_cm = tc.tile_pool(name="persist", bufs=1)
        persist = persist_cm.__enter__()

        wo_sb = []
        for k in range(HP):
            t = persist.tile([128, DIM], dt.bfloat16, tag=f"wo{k}", name=f"wo{k}")
            nc.sync.dma_start(out=t, in_=wo[k * 128:(k + 1) * 128, :])
            wo_sb.append(t)
        bias_sb = persist.tile([128, 3 * HP], dt.float32, tag="biases", name="biases")
        nc.sync.dma_start(out=bias_sb, in_=biases)

        qt_sb = [persist.tile([128, ROWS], dt.bfloat16, tag=f"qt{i}", name=f"qt{i}") for i in range(HP)]
        kt_sb = [persist.tile([128, ST], dt.bfloat16, tag=f"kt{i}", name=f"kt{i}") for i in range(HP)]
        v_sb = [persist.tile([128, VW], dt.bfloat16, tag=f"v{i}", name=f"v{i}") for i in range(NST)]
        ctx_sb = [persist.tile([128, ROWS], dt.bfloat16, tag=f"ctx{i}", name=f"ctx{i}") for i in range(HP)]

        # ---------------- phase A: projections ----------------
        with tc.tile_pool(name="projIn", bufs=1) as projin, \
             tc.tile_pool(name="psA", bufs=4, space="PSUM") as psA:
            xf_sb, xt_sb, wq_sb, wk_sb, wv_sb = [], [], [], [], []
            for k in range(HP):
                t = projin.tile([128, ROWS], dt.bfloat16, tag=f"xf{k}", name=f"xf{k}")
                nc.sync.dma_start(out=t, in_=xfT[k * 128:(k + 1) * 128, :])
                xf_sb.append(t)
                t = projin.tile([128, DIM], dt.bfloat16, tag=f"wq{k}", name=f"wq{k}")
                nc.sync.dma_start(out=t, in_=wq[k * 128:(k + 1) * 128, :])
                wq_sb.append(t)
            for k in range(HP):
                t = projin.tile([128, ST], dt.bfloat16, tag=f"xt{k}", name=f"xt{k}")
                nc.sync.dma_start(out=t, in_=xtT[k * 128:(k + 1) * 128, :])
                xt_sb.append(t)
                t = projin.tile([128, DIM], dt.bfloat16, tag=f"wk{k}", name=f"wk{k}")
                nc.sync.dma_start(out=t, in_=wk[k * 128:(k + 1) * 128, :])
                wk_sb.append(t)
                t = projin.tile([128, VW], dt.bfloat16, tag=f"wv{k}", name=f"wv{k}")
                nc.sync.dma_start(out=t, in_=wv[k * 128:(k + 1) * 128, :])
                wv_sb.append(t)
            wv_bias = projin.tile([1, VW], dt.bfloat16, tag="wvb", name="wvb")
            nc.sync.dma_start(out=wv_bias, in_=wv[DIM:DIM + 1, :])
            ones_sb = projin.tile([1, ST], dt.bfloat16, tag="ones", name="ones")
            nc.vector.memset(ones_sb, 1.0)

            # QT: per head-pair hp, [128, ROWS] = sum_k wq[k][:,hp]^T @ xf[k]
            for hp in range(HP):
                for n0 in range(0, ROWS, 512):
                    ps = psA.tile([128, 512], dt.float32, tag="psA", name="psA")
                    for k in range(HP):
                        nc.tensor.matmul(
                            ps, wq_sb[k][:, hp * 128:(hp + 1) * 128],
                            xf_sb[k][:, n0:n0 + 512],
                            start=(k == 0), stop=(k == HP - 1))
                    nc.vector.tensor_scalar_add(
                        out=qt_sb[hp][:, n0:n0 + 512], in0=ps,
                        scalar1=bias_sb[:, hp:hp + 1])
            # KT
            for hp in range(HP):
                for n0 in range(0, ST, 512):
                    ps = psA.tile([128, 512], dt.float32, tag="psA", name="psA")
                    for k in range(HP):
                        nc.tensor.matmul(
                            ps, wk_sb[k][:, hp * 128:(hp + 1) * 128],
                            xt_sb[k][:, n0:n0 + 512],
                            start=(k == 0), stop=(k == HP - 1))
                    nc.vector.tensor_scalar_add(
                        out=kt_sb[hp][:, n0:n0 + 512], in0=ps,
                        scalar1=bias_sb[:, HP + hp:HP + hp + 1])
            # V (+bias row +ones cols): [128st, VW] = xt_aug[:, st]^T^T... lhsT=xt chunks
            for st in range(NST):
                c0 = st * 128
                for n0, nw in ((0, 512), (512, VW - 512)):
                    ps = psA.tile([128, 512], dt.float32, tag="psA", name="psA")
                    for k in range(HP):
                        nc.tensor.matmul(
                            ps[:, :nw], xt_sb[k][:, c0:c0 + 128],
                            wv_sb[k][:, n0:n0 + nw],
                            start=(k == 0), stop=False)
                    nc.tensor.matmul(
                        ps[:, :nw], ones_sb[:, c0:c0 + 128],
                        wv_bias[:, n0:n0 + nw],
                        start=False, stop=True)
                    nc.vector.tensor_copy(out=v_sb[st][:, n0:n0 + nw], in_=ps[:, :nw])

        # mask tiles loaded after projIn closes (reuses freed SBUF)
        mask_sb = []
        for st in range(NST):
            t = persist.tile([128, ROWS], dt.bfloat16, tag=f"mask{st}", name=f"mask{st}")
            nc.sync.dma_start(out=t, in_=maskT[st * 128:(st + 1) * 128, :])
            mask_sb.append(t)

        # ---------------- phase B: attention ----------------
        # Head-PAIR processing with explicit tile_position packing:
        #  - scores: the two heads' K=64 matmuls row-packed at (0,0)/(64,0)
        #  - ctx:    the two heads' M=64 matmuls col-packed at (0,0)/(0,64)
        #            into ONE [128, ROWS] psum tile (head h rows 0:64, h' 64:128)
        #  - Z:      4x M=1 ones-matmuls col-packed at (0,32j)
        # Emission is software-pipelined (ctx for step k after scores for k+1)
        # so the in-order PE stream doesn't stall on the exp->mask chain.
        ones_col = persist.tile([128, 1], dt.bfloat16, tag="ones_col", name="ones_col")
        nc.vector.memset(ones_col, 1.0)
        ctxn = [persist.tile([128, ROWS], dt.bfloat16, tag=f"ctxn{i}", name=f"ctxn{i}")
                for i in range(HP)]
        ZJ = [(0, 0), (0, 512), (1, 0), (1, 512)]  # (h2, n0) per zps row 32j
        with tc.tile_pool(name="attn", bufs=6) as attn, \
             tc.tile_pool(name="z97", bufs=2) as z97p, \
             tc.tile_pool(name="z2", bufs=2) as z2p, \
             tc.tile_pool(name="rzbc", bufs=2) as rzbcp, \
             tc.tile_pool(name="psS", bufs=2, space="PSUM") as psS, \
             tc.tile_pool(name="psC", bufs=1, space="PSUM") as psC, \
             tc.tile_pool(name="psZ", bufs=1, space="PSUM") as psZ:

            pending = None
            ctxp_cur = None
            zps_cur = None

            def emit_ctx(hp, st, pp, ctxp, zps):
                for n0 in range(0, ROWS, 512):
                    for h2 in range(2):
                        nc.tensor.matmul(
                            ctxp[64 * h2:64 * h2 + 64, n0:n0 + 512],
                            v_sb[st][:, (2 * hp + h2) * HD:(2 * hp + h2 + 1) * HD],
                            pp[h2][:, n0:n0 + 512],
                            start=(st == 0), stop=(st == NST - 1),
                            tile_position=(0, 64 * h2))
                for j, (h2, n0) in enumerate(ZJ):
                    nc.tensor.matmul(
                        zps[32 * j:32 * j + 1, 0:512],
                        ones_col,
                        pp[h2][:, n0:n0 + 512],
                        start=(st == 0), stop=(st == NST - 1),
                        tile_position=(0, 32 * j))

            def drain_pair(hp, ctxp, zps):
                nc.vector.tensor_copy(out=ctx_sb[hp], in_=ctxp)
                z97 = z97p.tile([97, 512], dt.float32, tag="z97", name="z97")
                nc.vector.tensor_copy(out=z97, in_=zps[0:97, 0:512])
                z2 = z2p.tile([2, ROWS], dt.float32, tag="z2", name="z2")
                for j, (h2, n0) in enumerate(ZJ):
                    nc.sync.dma_start(out=z2[h2:h2 + 1, n0:n0 + 512],
                                      in_=z97[32 * j:32 * j + 1, :])
                rz2 = z2p.tile([2, ROWS], dt.float32, tag="rz2", name="rz2")
                nc.vector.reciprocal_approx_fast(out=rz2, in_=z2)
                rz2h = z2p.tile([2, ROWS], dt.bfloat16, tag="rz2h", name="rz2h")
                nc.vector.tensor_copy(out=rz2h, in_=rz2)
                nc.sync.dma_start(out=rz_dram[2 * hp:2 * hp + 2, :], in_=rz2h)
                bc = rzbcp.tile([128, ROWS], dt.bfloat16, tag="rzbc", name="rzbc")
                src = rz_dram[2 * hp:2 * hp + 2, :]
                bcast = bass.AP(tensor=src.tensor, offset=src.offset,
                                ap=[src.ap[0], [0, HD], src.ap[1]])
                nc.sync.dma_start(out=bc, in_=bcast)
                nc.vector.tensor_mul(out=ctxn[hp], in0=ctx_sb[hp], in1=bc)

            for hp in range(HP):
                for st in range(NST):
                    c0 = st * 128
                    if st == 0:
                        ctxp_cur = psC.tile([128, ROWS], dt.float32,
                                            tag="ctxp", name="ctxp")
                        zps_cur = psZ.tile([128, 512], dt.float32,
                                           tag="zps", name="zps")
                    sh = [psS.tile([128, ROWS], dt.float32, tag="sps", name="sps")
                          for _ in range(2)]
                    for n0 in range(0, ROWS, 512):
                        for h2 in range(2):
                            nc.tensor.matmul(
                                sh[h2][:, n0:n0 + 512],
                                kt_sb[hp][HD * h2:HD * h2 + HD, c0:c0 + 128],
                                qt_sb[hp][HD * h2:HD * h2 + HD, n0:n0 + 512],
                                start=True, stop=True,
                                tile_position=(64 * h2, 0))
                    pp = []
                    for h2 in range(2):
                        p = attn.tile([128, ROWS], dt.bfloat16, tag="p", name="p")
                        nc.scalar.activation(out=p, in_=sh[h2], func=EXP)
                        nc.vector.tensor_mul(out=p, in0=p, in1=mask_sb[st])
                        pp.append(p)
                    if pending is not None:
                        php, pst, ppp, pctxp, pzps = pending
                        emit_ctx(php, pst, ppp, pctxp, pzps)
                        if pst == NST - 1:
                            drain_pair(php, pctxp, pzps)
                    pending = (hp, st, pp, ctxp_cur, zps_cur)
            php, pst, ppp, pctxp, pzps = pending
            emit_ctx(php, pst, ppp, pctxp, pzps)
            drain_pair(php, pctxp, pzps)

        # ---------------- phase C: output projection ----------------
        with tc.tile_pool(name="outsb", bufs=2) as outsbp, \
             tc.tile_pool(name="psO", bufs=4, space="PSUM") as psO:
            for of in range(HP):
                o = outsbp.tile([128, ROWS], dt.float32, tag="outsb", name="outsb")
                for n0 in range(0, ROWS, 512):
                    ps = psO.tile([128, 512], dt.float32, tag="psO", name="psO")
                    for k in range(HP):
                        nc.tensor.matmul(
                            ps, wo_sb[k][:, of * 128:(of + 1) * 128],
                            ctxn[k][:, n0:n0 + 512],
                            start=(k == 0), stop=(k == HP - 1))
                    nc.vector.tensor_scalar_add(
                        out=o[:, n0:n0 + 512], in0=ps,
                        scalar1=bias_sb[:, 2 * HP + of:2 * HP + of + 1])
                nc.sync.dma_start(out=out[of * 128:(of + 1) * 128, :], in_=o)

        persist_cm.__exit__(None, None, None)

    nc.compile()
    return nc


def _get_nc():
    global _CACHED_NC
    if _CACHED_NC is None:
        _CACHED_NC = _build_nc()
    return _CACHED_NC


def _prep_inputs(from_tensor, to_tensor, attention_mask,
                 Wq, bq, Wk, bk, Wv, bv, Wo, bo):
    f32 = np.float32
    from_tensor = np.asarray(from_tensor, f32)
    to_tensor = np.asarray(to_tensor, f32)
    attention_mask = np.asarray(attention_mask)

    wq_h = (np.asarray(Wq, f32) * SCALE).astype(BF16)
    wk_h = np.asarray(Wk, f32).astype(BF16)
    wo_h = np.asarray(Wo, f32).astype(BF16)
    wv_aug = np.vstack([np.asarray(Wv, f32), np.asarray(bv, f32)[None, :]])
    wv_h = wv_aug.astype(BF16)

    biases = np.zeros((128, 3 * HP), f32)
    biases[:, 0:HP] = (np.asarray(bq, f32) * SCALE).reshape(HP, 128).T
    biases[:, HP:2 * HP] = np.asarray(bk, f32).reshape(HP, 128).T
    biases[:, 2 * HP:3 * HP] = np.asarray(bo, f32).reshape(HP, 128).T

    xtT_all = [np.ascontiguousarray(to_tensor[b].T).astype(BF16) for b in range(B)]

    in_maps = []
    for c in range(NCORES):
        b, half = c // 2, c % 2
        r0 = half * ROWS
        xfT = np.ascontiguousarray(from_tensor[b, r0:r0 + ROWS, :].T).astype(BF16)
        maskT = np.ascontiguousarray(
            attention_mask[b, r0:r0 + ROWS, :].T).astype(BF16)
        in_maps.append({
            "xfT": xfT, "xtT": xtT_all[b], "maskT": maskT,
            "wq": wq_h, "wk": wk_h, "wv": wv_h, "wo": wo_h, "biases": biases,
        })
    return in_maps


def _assemble(results):
    out = np.empty((B, SF, DIM), np.float32)
    for c, r in enumerate(results):
        b, half = c // 2, c % 2
        r0 = half * ROWS
        out[b, r0:r0 + ROWS, :] = np.asarray(r["out"], np.float32).T
    return out


def _run(in_maps, trace=False):
    from concourse.bass_utils import run_bass_kernel_spmd
    nc = _get_nc()
    return run_bass_kernel_spmd(nc, in_maps, core_ids=list(range(NCORES)),
                                trace=trace)


def kernel(**inputs):
    in_maps = _prep_inputs(**inputs)
    res = _run(in_maps, trace=False)
    return _assemble(res.results)


def kernel_profiled(**inputs):
    """Returns (output, exec_time_ns, trace_path)."""
    in_maps = _prep_inputs(**inputs)
    res = _run(in_maps, trace=True)
    trace_path = None
    if res.instructions_and_trace is not None:
        trace_path = res.instructions_and_trace[1]
    return _assemble(res.results), res.exec_time_ns, trace_path


# revision 10
# speedup vs baseline: 1.4565x; 1.0667x over previous
"""Multi-head attention on 8 TRN2 NeuronCores (Bass/Tile, SPMD, no collectives).

Problem: B=4, Sf=St=2048, DIM=768, H=12, Dh=64, f32 reference.

Sharding: (batch, Sf/2) -> 8 shards. Core c handles batch b=c//2, query rows
[512*(c%2)*2 : +1024). K/V projections for a batch are recomputed by both cores
of the pair (cheaper than any collective).

Device dataflow is fully transposed so no on-chip transposes are needed:
  QT[h]  [64,1024]  = Wq_h^T @ xf^T        (lhsT=Wq cols, rhs=xf^T)
  KT[h]  [64,2048]  = Wk_h^T @ xt^T
  V      [2048,768] = xt_aug^T @ Wv_aug   (bias row via K=1 matmul)
  S^T    [St,Sq]    = KT_chunk^T^T ... per (head, st-tile): lhsT=KT[64,128], rhs=QT[64,1024]
  P^T    = exp(S^T) * mask^T               (no max-subtract; masked lanes exp->*0)
  ctx^T  [128,1024] = V^T @ P^T, two heads col-packed via tile_position;
  Z      via 4x col-packed M=1 ones-matmuls  (both accumulated over st in PSUM)
  out^T  [768,1024] = Wo^T @ (ctx^T * 1/Z) + bo
Host transposes out^T back and stitches the 8 shards.
"""

import os
import numpy as np
import ml_dtypes

BF16 = ml_dtypes.bfloat16

B, SF, ST, DIM = 4, 2048, 2048, 768
NH, HD = 12, 64
SCALE = HD ** -0.5
NCORES = 8
ROWS = B * SF // NCORES      # 1024 query rows per core
HP = NH // 2                 # 6 head-pairs == 6 x 128-partition chunks of DIM
VW = DIM                     # 768: V width (no ones columns; Z via packed M=1 matmuls)
NST = ST // 128              # 16 st tiles

_CACHED_NC = None


def _build_nc():
    from concourse import bacc, tile, mybir
    import concourse.bass as bass

    dt = mybir.dt
    nc = bacc.Bacc("TRN2", target_bir_lowering=False, debug=False,
                   num_devices=NCORES)

    xfT = nc.dram_tensor("xfT", [DIM, ROWS], dt.bfloat16, kind="ExternalInput").ap()
    xtT = nc.dram_tensor("xtT", [DIM, ST], dt.bfloat16, kind="ExternalInput").ap()
    maskT = nc.dram_tensor("maskT", [ST, ROWS], dt.bfloat16, kind="ExternalInput").ap()
    wq = nc.dram_tensor("wq", [DIM, DIM], dt.bfloat16, kind="ExternalInput").ap()
    wk = nc.dram_tensor("wk", [DIM, DIM], dt.bfloat16, kind="ExternalInput").ap()
    wv = nc.dram_tensor("wv", [DIM + 1, VW], dt.bfloat16, kind="ExternalInput").ap()
    wo = nc.dram_tensor("wo", [DIM, DIM], dt.bfloat16, kind="ExternalInput").ap()
    biases = nc.dram_tensor("biases", [128, 3 * HP], dt.float32, kind="ExternalInput").ap()
    out = nc.dram_tensor("out", [DIM, ROWS], dt.float32, kind="ExternalOutput").ap()
    rz_dram = nc.dram_tensor("rz_scratch", [NH, ROWS], dt.bfloat16).ap()

    EXP = mybir.ActivationFunctionType.Exp

    with tile.TileContext(nc) as tc:
        persist_cm = tc.tile_pool(name="persist", bufs=1)
        persist = persist_cm.__enter__()

        wo_sb = []
        for k in range(HP):
            t = persist.tile([128, DIM], dt.bfloat16, tag=f"wo{k}", name=f"wo{k}")
            nc.sync.dma_start(out=t, in_=wo[k * 128:(k + 1) * 128, :])
            wo_sb.append(t)
        bias_sb = persist.tile([128, 3 * HP], dt.float32, tag="biases", name="biases")
        nc.sync.dma_start(out=bias_sb, in_=biases)

        qt_sb = [persist.tile([128, ROWS], dt.bfloat16, tag=f"qt{i}", name=f"qt{i}") for i in range(HP)]
        kt_sb = [persist.tile([128, ST], dt.bfloat16, tag=f"kt{i}", name=f"kt{i}") for i in range(HP)]
        v_sb = [persist.tile([128, VW], dt.bfloat16, tag=f"v{i}", name=f"v{i}") for i in range(NST)]
        ctx_sb = [persist.tile([128, ROWS], dt.bfloat16, tag=f"ctx{i}", name=f"ctx{i}") for i in range(HP)]

        # ---------------- phase A: projections ----------------
        with tc.tile_pool(name="projIn", bufs=1) as projin, \
             tc.tile_pool(name="psA", bufs=4, space="PSUM") as psA:
            xf_sb, xt_sb, wq_sb, wk_sb, wv_sb = [], [], [], [], []
            for k in range(HP):
                t = projin.tile([128, ROWS], dt.bfloat16, tag=f"xf{k}", name=f"xf{k}")
                nc.sync.dma_start(out=t, in_=xfT[k * 128:(k + 1) * 128, :])
                xf_sb.append(t)
                t = projin.tile([128, DIM], dt.bfloat16, tag=f"wq{k}", name=f"wq{k}")
                nc.sync.dma_start(out=t, in_=wq[k * 128:(k + 1) * 128, :])
                wq_sb.append(t)
            for k in range(HP):
                t = projin.tile([128, ST], dt.bfloat16, tag=f"xt{k}", name=f"xt{k}")
                nc.sync.dma_start(out=t, in_=xtT[k * 128:(k + 1) * 128, :])
                xt_sb.append(t)
                t = projin.tile([128, DIM], dt.bfloat16, tag=f"wk{k}", name=f"wk{k}")
                nc.sync.dma_start(out=t, in_=wk[k * 128:(k + 1) * 128, :])
                wk_sb.append(t)
                t = projin.tile([128, VW], dt.bfloat16, tag=f"wv{k}", name=f"wv{k}")
                nc.sync.dma_start(out=t, in_=wv[k * 128:(k + 1) * 128, :])
                wv_sb.append(t)
            wv_bias = projin.tile([1, VW], dt.bfloat16, tag="wvb", name="wvb")
            nc.sync.dma_start(out=wv_bias, in_=wv[DIM:DIM + 1, :])
            ones_sb = projin.tile([1, ST], dt.bfloat16, tag="ones", name="ones")
            nc.vector.memset(ones_sb, 1.0)

            # QT: per head-pair hp, [128, ROWS] = sum_k wq[k][:,hp]^T @ xf[k]
            for hp in range(HP):
                for n0 in range(0, ROWS, 512):
                    ps = psA.tile([128, 512], dt.float32, tag="psA", name="psA")
                    for k in range(HP):
                        nc.tensor.matmul(
                            ps, wq_sb[k][:, hp * 128:(hp + 1) * 128],
                            xf_sb[k][:, n0:n0 + 512],
                            start=(k == 0), stop=(k == HP - 1))
                    nc.vector.tensor_scalar_add(
                        out=qt_sb[hp][:, n0:n0 + 512], in0=ps,
                        scalar1=bias_sb[:, hp:hp + 1])
            # KT
            for hp in range(HP):
                for n0 in range(0, ST, 512):
                    ps = psA.tile([128, 512], dt.float32, tag="psA", name="psA")
                    for k in range(HP):
                        nc.tensor.matmul(
                            ps, wk_sb[k][:, hp * 128:(hp + 1) * 128],
                            xt_sb[k][:, n0:n0 + 512],
                            start=(k == 0), stop=(k == HP - 1))
                    nc.vector.tensor_scalar_add(
                        out=kt_sb[hp][:, n0:n0 + 512], in0=ps,
                        scalar1=bias_sb[:, HP + hp:HP + hp + 1])
            # V (+bias row +ones cols): [128st, VW] = xt_aug[:, st]^T^T... lhsT=xt chunks
            for st in range(NST):
                c0 = st * 128
                for n0, nw in ((0, 512), (512, VW - 512)):
                    ps = psA.tile([128, 512], dt.float32, tag="psA", name="psA")
                    for k in range(HP):
                        nc.tensor.matmul(
                            ps[:, :nw], xt_sb[k][:, c0:c0 + 128],
                            wv_sb[k][:, n0:n0 + nw],
                            start=(k == 0), stop=False)
                    nc.tensor.matmul(
                        ps[:, :nw], ones_sb[:, c0:c0 + 128],
                        wv_bias[:, n0:n0 + nw],
                        start=False, stop=True)
                    nc.vector.tensor_copy(out=v_sb[st][:, n0:n0 + nw], in_=ps[:, :nw])

        # mask tiles loaded after projIn closes (reuses freed SBUF)
        mask_sb = []
        for st in range(NST):
            t = persist.tile([128, ROWS], dt.bfloat16, tag=f"mask{st}", name=f"mask{st}")
            nc.sync.dma_start(out=t, in_=maskT[st * 128:(st + 1) * 128, :])
            mask_sb.append(t)

        # ---------------- phase B: attention ----------------
        # Head-PAIR processing with explicit tile_position packing:
        #  - scores: the two heads' K=64 matmuls row-packed at (0,0)/(64,0)
        #  - ctx:    the two heads' M=64 matmuls col-packed at (0,0)/(0,64)
        #            into ONE [128, ROWS] psum tile (head h rows 0:64, h' 64:128)
        #  - Z:      4x M=1 ones-matmuls col-packed at (0,32j)
        # Emission is software-pipelined (ctx for step k after scores for k+1)
        # so the in-order PE stream doesn't stall on the exp->mask chain.
        ones_col = persist.tile([128, 1], dt.bfloat16, tag="ones_col", name="ones_col")
        nc.vector.memset(ones_col, 1.0)
        ctxn = [persist.tile([128, ROWS], dt.bfloat16, tag=f"ctxn{i}", name=f"ctxn{i}")
                for i in range(HP)]
        # zps row 32j holds Z for (sq-half ni, head h2), j = 2*ni + h2
        ZJ = [(0, 0), (1, 0), (0, 512), (1, 512)]  # (h2, n0) per zps row 32j
        with tc.tile_pool(name="attn", bufs=6) as attn, \
             tc.tile_pool(name="z97", bufs=2) as z97p, \
             tc.tile_pool(name="z2", bufs=2) as z2p, \
             tc.tile_pool(name="rzbc", bufs=2) as rzbcp, \
             tc.tile_pool(name="psS", bufs=2, space="PSUM") as psS, \
             tc.tile_pool(name="psC", bufs=1, space="PSUM") as psC, \
             tc.tile_pool(name="psZ", bufs=1, space="PSUM") as psZ:

            pending = None
            ctxp_cur = None
            zps_cur = None

            def emit_ctx(hp, st, pp, ctxp, zps):
                # pp[ni] holds [P_h0 sq-half ni | P_h1 sq-half ni] on the free axis
                for ni in range(2):
                    n0 = 512 * ni
                    for h2 in range(2):
                        nc.tensor.matmul(
                            ctxp[64 * h2:64 * h2 + 64, n0:n0 + 512],
                            v_sb[st][:, (2 * hp + h2) * HD:(2 * hp + h2 + 1) * HD],
                            pp[ni][:, 512 * h2:512 * h2 + 512],
                            start=(st == 0), stop=(st == NST - 1),
                            tile_position=(0, 64 * h2))
                for j, (h2, n0) in enumerate(ZJ):
                    ni = n0 // 512
                    nc.tensor.matmul(
                        zps[32 * j:32 * j + 1, 0:512],
                        ones_col,
                        pp[ni][:, 512 * h2:512 * h2 + 512],
                        start=(st == 0), stop=(st == NST - 1),
                        tile_position=(0, 32 * j))

            def drain_pair(hp, ctxp, zps):
                nc.vector.tensor_copy(out=ctx_sb[hp], in_=ctxp)
                z97 = z97p.tile([97, 512], dt.float32, tag="z97", name="z97")
                nc.vector.tensor_copy(out=z97, in_=zps[0:97, 0:512])
                z2 = z2p.tile([2, ROWS], dt.float32, tag="z2", name="z2")
                for j, (h2, n0) in enumerate(ZJ):
                    nc.sync.dma_start(out=z2[h2:h2 + 1, n0:n0 + 512],
                                      in_=z97[32 * j:32 * j + 1, :])
                rz2 = z2p.tile([2, ROWS], dt.float32, tag="rz2", name="rz2")
                nc.vector.reciprocal_approx_fast(out=rz2, in_=z2)
                rz2h = z2p.tile([2, ROWS], dt.bfloat16, tag="rz2h", name="rz2h")
                nc.vector.tensor_copy(out=rz2h, in_=rz2)
                nc.sync.dma_start(out=rz_dram[2 * hp:2 * hp + 2, :], in_=rz2h)
                bc = rzbcp.tile([128, ROWS], dt.bfloat16, tag="rzbc", name="rzbc")
                src = rz_dram[2 * hp:2 * hp + 2, :]
                bcast = bass.AP(tensor=src.tensor, offset=src.offset,
                                ap=[src.ap[0], [0, HD], src.ap[1]])
                nc.sync.dma_start(out=bc, in_=bcast)
                nc.vector.tensor_mul(out=ctxn[hp], in0=ctx_sb[hp], in1=bc)

            for hp in range(HP):
                for st in range(NST):
                    c0 = st * 128
                    if st == 0:
                        ctxp_cur = psC.tile([128, ROWS], dt.float32,
                                            tag="ctxp", name="ctxp")
                        zps_cur = psZ.tile([128, 512], dt.float32,
                                           tag="zps", name="zps")
                    # sps tile ni = [scores_h0 sq-half ni | scores_h1 sq-half ni]:
                    # both row-packed pair members land in ONE tile, so their
                    # slot dependencies resolve together and pairs never break.
                    pp = []
                    for ni in range(2):
                        n0 = 512 * ni
                        sps = psS.tile([128, ROWS], dt.float32, tag="sps", name="sps")
                        for h2 in range(2):
                            nc.tensor.matmul(
                                sps[:, 512 * h2:512 * h2 + 512],
                                kt_sb[hp][HD * h2:HD * h2 + HD, c0:c0 + 128],
                                qt_sb[hp][HD * h2:HD * h2 + HD, n0:n0 + 512],
                                start=True, stop=True,
                                tile_position=(64 * h2, 0))
                        p = attn.tile([128, ROWS], dt.bfloat16, tag="p", name="p")
                        nc.scalar.activation(out=p, in_=sps, func=EXP)
                        for h2 in range(2):
                            nc.vector.tensor_mul(
                                out=p[:, 512 * h2:512 * h2 + 512],
                                in0=p[:, 512 * h2:512 * h2 + 512],
                                in1=mask_sb[st][:, n0:n0 + 512])
                        pp.append(p)
                    if pending is not None:
                        php, pst, ppp, pctxp, pzps = pending
                        emit_ctx(php, pst, ppp, pctxp, pzps)
                        if pst == NST - 1:
                            drain_pair(php, pctxp, pzps)
                    pending = (hp, st, pp, ctxp_cur, zps_cur)
            php, pst, ppp, pctxp, pzps = pending
            emit_ctx(php, pst, ppp, pctxp, pzps)
            drain_pair(php, pctxp, pzps)

        # ---------------- phase C: output projection ----------------
        with tc.tile_pool(name="outsb", bufs=2) as outsbp, \
             tc.tile_pool(name="psO", bufs=4, space="PSUM") as psO:
            for of in range(HP):
                o = outsbp.tile([128, ROWS], dt.float32, tag="outsb", name="outsb")
                for n0 in range(0, ROWS, 512):
                    ps = psO.tile([128, 512], dt.float32, tag="psO", name="psO")
                    for k in range(HP):
                        nc.tensor.matmul(
                            ps, wo_sb[k][:, of * 128:(of + 1) * 128],
                            ctxn[k][:, n0:n0 + 512],
                            start=(k == 0), stop=(k == HP - 1))
                    nc.vector.tensor_scalar_add(
                        out=o[:, n0:n0 + 512], in0=ps,
                        scalar1=bias_sb[:, 2 * HP + of:2 * HP + of + 1])
                nc.sync.dma_start(out=out[of * 128:(of + 1) * 128, :], in_=o)

        persist_cm.__exit__(None, None, None)

    nc.compile()
    return nc


def _get_nc():
    global _CACHED_NC
    if _CACHED_NC is None:
        _CACHED_NC = _build_nc()
    return _CACHED_NC


def _prep_inputs(from_tensor, to_tensor, attention_mask,
                 Wq, bq, Wk, bk, Wv, bv, Wo, bo):
    f32 = np.float32
    from_tensor = np.asarray(from_tensor, f32)
    to_tensor = np.asarray(to_tensor, f32)
    attention_mask = np.asarray(attention_mask)

    wq_h = (np.asarray(Wq, f32) * SCALE).astype(BF16)
    wk_h = np.asarray(Wk, f32).astype(BF16)
    wo_h = np.asarray(Wo, f32).astype(BF16)
    wv_aug = np.vstack([np.asarray(Wv, f32), np.asarray(bv, f32)[None, :]])
    wv_h = wv_aug.astype(BF16)

    biases = np.zeros((128, 3 * HP), f32)
    biases[:, 0:HP] = (np.asarray(bq, f32) * SCALE).reshape(HP, 128).T
    biases[:, HP:2 * HP] = np.asarray(bk, f32).reshape(HP, 128).T
    biases[:, 2 * HP:3 * HP] = np.asarray(bo, f32).reshape(HP, 128).T

    xtT_all = [np.ascontiguousarray(to_tensor[b].T).astype(BF16) for b in range(B)]

    in_maps = []
    for c in range(NCORES):
        b, half = c // 2, c % 2
        r0 = half * ROWS
        xfT = np.ascontiguousarray(from_tensor[b, r0:r0 + ROWS, :].T).astype(BF16)
        maskT = np.ascontiguousarray(
            attention_mask[b, r0:r0 + ROWS, :].T).astype(BF16)
        in_maps.append({
            "xfT": xfT, "xtT": xtT_all[b], "maskT": maskT,
            "wq": wq_h, "wk": wk_h, "wv": wv_h, "wo": wo_h, "biases": biases,
        })
    return in_maps


def _assemble(results):
    out = np.empty((B, SF, DIM), np.float32)
    for c, r in enumerate(results):
        b, half = c // 2, c % 2
        r0 = half * ROWS
        out[b, r0:r0 + ROWS, :] = np.asarray(r["out"], np.float32).T
    return out


def _run(in_maps, trace=False):
    from concourse.bass_utils import run_bass_kernel_spmd
    nc = _get_nc()
    return run_bass_kernel_spmd(nc, in_maps, core_ids=list(range(NCORES)),
                                trace=trace)


def kernel(**inputs):
    in_maps = _prep_inputs(**inputs)
    res = _run(in_maps, trace=False)
    return _assemble(res.results)


def kernel_profiled(**inputs):
    """Returns (output, exec_time_ns, trace_path)."""
    in_maps = _prep_inputs(**inputs)
    res = _run(in_maps, trace=True)
    trace_path = None
    if res.instructions_and_trace is not None:
        trace_path = res.instructions_and_trace[1]
    return _assemble(res.results), res.exec_time_ns, trace_path
